# revision 1
# baseline (speedup 1.0000x reference)
"""Trainium2 Bass kernel for nn_BiomechanicsLoss (masked quadratic-form loss).

Math (per point): et = [u0, v1, w2, .5(u1+v0), .5(u2+w0), .5(w1+v2)],
q = et^T C et with C = inv(compliance) cast to f32.  Loss =
sqrt(sum_masked(q^2)) / count_masked, mask = gt_sdf < 1e-8.

Because q = et^T C et == et^T sym(C) et and C is block-diagonal
(3x3 normal block + diagonal shear block), with Fm = diag(1,1,1,.5,.5,.5):
  q = w11*s1^2 + w22*s2^2 + w33*s3^2 + w12*s1*s2 + w13*s1*s3 + w23*s2*s3
      + d*(s4^2 + s5^2 + s6^2)
where s1..s3 = u0, v1, w2 ; s4 = u1+v0 ; s5 = u2+w0 ; s6 = w1+v2 and the
weights come from M = Fm*sym(C)*Fm (all positive for these constants).

Sharding: pure data-parallel over the N point dimension across 8 cores; each
core reduces its 524288-point shard to per-partition partials [128, 2*NT]
(per-chunk sum(mask*q^2) and count columns); the host sums 8*128*NT partials,
takes sqrt and divides.

The host packs each core's shard chunk-major and component-separated
([u0|v1|w2|u1|v0|u2|w0|w1|v2|sd] per chunk, partition-major inside each
block).  That makes every chunk ONE contiguous 2-4MB DMA (~97% of the
358GB/s per-core HBM roofline) and every SBUF read contiguous (no stride-3
penalty, wide fused ops).  Per chunk (F points/partition):
  VectorE: 3 f32 shear adds, mask via tensor_scalar(is_lt) with fused
           row-sum accum (= count, free), cross products factored as
           p1*(p2+p3) + p2*p3 on pre-scaled bf16 copies (2x mode), a
           3-level wide bf16 fold of the 8 weighted terms, q*m
  ScalarE: pre-scaled copies p12|p3 (alpha-factorization of the cross
           weights, a1==a2 so u0|v1 share one wide copy), weighted squares
           as wide activation(Square, scale) ops, final Square(q*m) with
           accum_out -> per-partition sum(mask*q^2)
  chunks tapered [512,1024,1024,1024,512] so the first compute starts early
  and the final serial chain after the last DMA is short.
Measured ~78.7us/core on TRN2 vs ~56us pure-DMA roofline (fixed NEFF
preamble + drain/barrier tail account for most of the difference).
"""

import numpy as np

N = 4_194_304
NCORES = 8
N_LOCAL = N // NCORES  # 524288
P = 128
J = N_LOCAL // P  # 4096 points per partition (partition-major layout)
# chunk widths; tapered head (compute starts sooner) and tail (short final
# serial chain)
CHUNKS = [512, 1024, 1024, 1024, 512]
NT = len(CHUNKS)
assert sum(CHUNKS) == J

THRESH = 1e-8


def _weights():
    vp, Ep = 0.4, 0.21
    Ci = np.zeros((6, 6), dtype=np.float64)
    Ci[0, 0] = 1 / Ep;  Ci[0, 1] = -vp / Ep; Ci[0, 2] = -vp / Ep
    Ci[1, 0] = -vp / Ep; Ci[1, 1] = 1 / Ep;  Ci[1, 2] = -vp / Ep
    Ci[2, 0] = -vp;      Ci[2, 1] = -vp;     Ci[2, 2] = 1 / Ep
    Ci[3, 3] = 2 * (1 + vp) / Ep
    Ci[4, 4] = Ci[3, 3]
    Ci[5, 5] = Ci[3, 3]
    # match reference: inverse computed in f64, cast to f32
    C = np.linalg.inv(Ci).astype(np.float32).astype(np.float64)
    Cs = 0.5 * (C + C.T)
    A = Cs[:3, :3]
    d = 0.25 * Cs[3, 3]
    return dict(
        w11=A[0, 0], w22=A[1, 1], w33=A[2, 2],
        w12=2 * A[0, 1], w13=2 * A[0, 2], w23=2 * A[1, 2],
        d=d,
    )


_NC = None


def _build_nc():
    import concourse.bacc as bacc
    import concourse.mybir as mybir
    import concourse.tile as tile

    W = _weights()
    rd = float(np.sqrt(W["d"]))
    # factor cross weights: w12 = a1*a2, w13 = a1*a3, w23 = a2*a3 so the
    # cross products use pre-scaled bf16 copies p_i = a_i*s_i (all-bf16 ->
    # DVE 2x mode); a1 == a2 and w11 == w22 for these constants, so u0|v1
    # share one wide scaled copy and one wide square.
    a1s = float(np.sqrt(W["w12"] * W["w13"] / W["w23"]))
    a2s = float(W["w12"] / a1s)
    a3s = float(W["w13"] / a1s)
    assert abs(a1s - a2s) < 1e-12 and abs(W["w11"] - W["w22"]) < 1e-12
    rz12f = float(np.sqrt(W["w11"]) / a1s)  # z12 = Sq(p12 * rz12f)
    rz3f = float(np.sqrt(W["w33"]) / a3s)   # z3  = Sq(p3 * rz3f)

    f32 = mybir.dt.float32
    bf16 = mybir.dt.bfloat16
    Sq = mybir.ActivationFunctionType.Square
    ALU = mybir.AluOpType

    nc = bacc.Bacc()
    # host packs each core's shard chunk-major: for each chunk t, partition p:
    # [u (3F interleaved) | v (3F) | w (3F) | sd (F)] -> one contiguous DMA
    # per chunk (4MB-class, ~97% DMA efficiency)
    packed = nc.dram_tensor("packed", [P, 10 * J], f32, kind="ExternalInput")
    out = nc.dram_tensor("out", [P, 2 * NT], f32, kind="ExternalOutput")

    with tile.TileContext(nc) as tc:
        with (
            tc.tile_pool(name="io", bufs=2) as io,
            tc.tile_pool(name="mid", bufs=3) as mid,
            tc.tile_pool(name="stats", bufs=1) as stats_pool,
        ):
            stats = stats_pool.tile([P, 2 * NT], f32)

            c0 = 0
            for t, F in enumerate(CHUNKS):
                buf = io.tile([P, 10 * F], f32, tag="buf")
                nc.sync.dma_start(out=buf[:], in_=packed[:, c0:c0 + 10 * F])
                c0 += 10 * F

                # host-packed chunk layout (all contiguous [P, F] blocks):
                # [u0 v1 w2 | u1 v0 u2 w0 w1 v2 | sd]
                u0v1 = buf[:, 0 * F:2 * F]
                w2 = buf[:, 2 * F:3 * F]
                u1, v0 = buf[:, 3 * F:4 * F], buf[:, 4 * F:5 * F]
                u2, w0 = buf[:, 5 * F:6 * F], buf[:, 6 * F:7 * F]
                w1, v2 = buf[:, 7 * F:8 * F], buf[:, 8 * F:9 * F]
                sd = buf[:, 9 * F:10 * F]

                # shear strain components into one [P,3F] tile
                # (f32 contiguous in, bf16 out)
                s456 = mid.tile([P, 3 * F], bf16, tag="s456")
                nc.vector.tensor_add(s456[:, 0:F], u1, v0)
                nc.vector.tensor_add(s456[:, F:2 * F], u2, w0)
                nc.vector.tensor_add(s456[:, 2 * F:3 * F], w1, v2)

                # pre-scaled bf16 copies on ScalarE (alpha1 == alpha2, so
                # u0 and v1 share one 2F-wide copy)
                p12 = mid.tile([P, 2 * F], bf16, tag="p12")
                p3 = mid.tile([P, F], bf16, tag="p3")
                nc.scalar.mul(p12, u0v1, a1s)
                nc.scalar.mul(p3, w2, a3s)

                # mask (f32 single-src 2x); fused row-sum accum = count
                m = mid.tile([P, F], bf16, tag="m")
                nc.vector.tensor_scalar(
                    out=m, in0=sd, scalar1=THRESH, scalar2=None, op0=ALU.is_lt,
                    op1=ALU.add, accum_out=stats[:, NT + t:NT + t + 1])

                # term tiles: X = [z4 z5 z6 | z3], Y1 = [z1 z2], Y2 = [ca cb]
                X = mid.tile([P, 4 * F], bf16, tag="X")
                Y1 = mid.tile([P, 2 * F], bf16, tag="Y1")
                Y2 = mid.tile([P, 2 * F], bf16, tag="Y2")

                # weighted squares on ScalarE (wide ops; shared scales)
                nc.scalar.activation(X[:, 0:3 * F], s456, Sq, scale=rd)
                nc.scalar.activation(X[:, 3 * F:4 * F], p3, Sq, scale=rz3f)
                nc.scalar.activation(Y1, p12, Sq, scale=rz12f)

                # cross products, factored: p1p2 + p1p3 + p2p3 =
                # p1*(p2+p3) + p2*p3  (all bf16, DVE 2x)
                tp = mid.tile([P, F], bf16, tag="tp")
                nc.vector.tensor_add(tp, p12[:, F:2 * F], p3)
                nc.vector.tensor_mul(Y2[:, 0:F], p12[:, 0:F], tp)
                nc.vector.tensor_mul(Y2[:, F:2 * F], p12[:, F:2 * F], p3)

                # combine 8 terms with a 3-level wide fold (work 7F, 4 ops)
                nc.vector.tensor_add(Y1, Y1, Y2)                    # 2F
                nc.vector.tensor_add(X[:, 0:2 * F], X[:, 0:2 * F],
                                     X[:, 2 * F:4 * F])             # 2F
                nc.vector.tensor_add(Y1, Y1, X[:, 0:2 * F])         # 2F
                q = p3  # reuse consumed tile for q
                nc.vector.tensor_add(q, Y1[:, 0:F], Y1[:, F:2 * F])  # F

                # qm = q * mask (bf16 2x), then ssq via fused square+row-sum
                nc.vector.tensor_mul(m, q, m)
                junk1 = mid.tile([P, F], bf16, tag="junk1")
                nc.scalar.activation(
                    junk1, m, Sq, accum_out=stats[:, t:t + 1])

            nc.sync.dma_start(out=out[:, :], in_=stats[:])

    nc.compile()
    return nc


def _get_nc():
    global _NC
    if _NC is None:
        _NC = _build_nc()
    return _NC


def _run(in_maps, trace=False, **kwargs):
    from concourse.bass_utils import run_bass_kernel_spmd

    nc = _get_nc()
    return run_bass_kernel_spmd(
        nc, in_maps, core_ids=list(range(NCORES)), trace=trace, **kwargs)


def _make_in_maps(grad_u, grad_v, grad_w, gt_sdf):
    grad_u = np.asarray(grad_u, dtype=np.float32)
    grad_v = np.asarray(grad_v, dtype=np.float32)
    grad_w = np.asarray(grad_w, dtype=np.float32)
    gt_sdf = np.asarray(gt_sdf, dtype=np.float32)
    in_maps = []
    for c in range(NCORES):
        sl = slice(c * N_LOCAL, (c + 1) * N_LOCAL)
        gu = grad_u[sl].reshape(P, J, 3)
        gv = grad_v[sl].reshape(P, J, 3)
        gw = grad_w[sl].reshape(P, J, 3)
        sd = gt_sdf[sl].reshape(P, J)
        parts = []
        off = 0
        for F in CHUNKS:
            s = slice(off, off + F)
            parts += [gu[:, s, 0], gv[:, s, 1], gw[:, s, 2],
                      gu[:, s, 1], gv[:, s, 0],
                      gu[:, s, 2], gw[:, s, 0],
                      gw[:, s, 1], gv[:, s, 2],
                      sd[:, s]]
            off += F
        packed = np.ascontiguousarray(np.concatenate(parts, axis=1))
        in_maps.append({"packed": packed})
    return in_maps


def _finalize(results):
    ssq = 0.0
    cnt = 0.0
    for res in results:
        st = np.asarray(res["out"], dtype=np.float64)
        ssq += st[:, :NT].sum()
        cnt += st[:, NT:].sum()
    Wv = np.sqrt(ssq)
    return np.float32(Wv / cnt)


def kernel(grad_u, grad_v, grad_w, gt_sdf):
    in_maps = _make_in_maps(grad_u, grad_v, grad_w, gt_sdf)
    res = _run(in_maps, trace=False)
    return _finalize(res.results)



# revision 5
# speedup vs baseline: 1.2120x; 1.2120x over previous
"""Trainium2 Bass kernel for nn_BiomechanicsLoss (masked quadratic-form loss).

Math per point: et = [u0, v1, w2, .5(u1+v0), .5(u2+w0), .5(w1+v2)],
q = et^T C et (C = f32 stiffness), loss = sqrt(sum_masked(q^2)) / count,
mask = gt_sdf < 1e-8.

q decomposes (A = 3x3 normal block of sym(C), d = Cs[3,3]/4) as
  q = alpha*(b1+b2+b3)^2 + g1*(b1^2+b2^2) + 1.0*b3^2 + x4+x5+x6
with b_i = a_i * s_i (host-applied scale), x_k = d*s_k'^2 (sqrt(d) host-
applied), and alpha solved so gamma3 == 1 exactly: b3^2 then shares the
scale-1 wide Square with the shear terms.

Distribution: pure data-parallel over points, 8 cores; per-core shard is
host-packed to bf16 [128, 10*J] with all constant scales folded into the
quantization; each core reduces to per-chunk (ssq, count) columns which the
host combines.

Per chunk of width F (points/partition) the device does:
  DMA:    D1 [A4 A5 A6 | b3] -> S[0:4F]   (HWDGE)
          D2 [B4 B5 B6] +=    S[0:3F]     (SWDGE accum add: shear sums free)
          D3 [b1 b2 | sd] ->  C[0:3F]     (HWDGE)
  ScalarE: X[0:3F]=Sq(S[0:3F]); Z[0:2F]=Sq(rg1*b12); Z[2F:3F]=Sq(ra*s0);
           Sq(qm) accum -> ssq column
  VectorE: z3 = b3*b3 -> X[3F:4F]; s0 = b1+b2+b3 (2 adds);
           fold X += Z, X[0:2F] += X[2F:4F], q = X0+X1;
           mask via tensor_scalar(is_lt) with count accum; qm = q*m
"""

import numpy as np

N = 4_194_304
NCORES = 8
N_LOCAL = N // NCORES  # 524288
P = 128
J = N_LOCAL // P  # 4096 points per partition
CHUNKS = [512, 1536, 1536, 512]
NT = len(CHUNKS)
assert sum(CHUNKS) == J

THRESH = 1e-8


def _weights():
    vp, Ep = 0.4, 0.21
    Ci = np.zeros((6, 6), dtype=np.float64)
    Ci[0, 0] = 1 / Ep;  Ci[0, 1] = -vp / Ep; Ci[0, 2] = -vp / Ep
    Ci[1, 0] = -vp / Ep; Ci[1, 1] = 1 / Ep;  Ci[1, 2] = -vp / Ep
    Ci[2, 0] = -vp;      Ci[2, 1] = -vp;     Ci[2, 2] = 1 / Ep
    Ci[3, 3] = 2 * (1 + vp) / Ep
    Ci[4, 4] = Ci[3, 3]
    Ci[5, 5] = Ci[3, 3]
    # match reference: inverse computed in f64, cast to f32
    C = np.linalg.inv(Ci).astype(np.float32).astype(np.float64)
    Cs = 0.5 * (C + C.T)
    A = Cs[:3, :3]
    d = 0.25 * Cs[3, 3]
    A12, A13 = A[0, 1], A[0, 2]
    # alpha s.t. gamma3 == 1 (A13 == A23, A11 == A22 for these constants)
    alpha = A13 ** 2 / (A[2, 2] * A12 - A13 ** 2)
    a1 = np.sqrt(A12 / alpha)
    a3 = a1 * A13 / A12
    g1 = A[0, 0] / a1 ** 2 - alpha
    return dict(
        rd=float(np.sqrt(d)), a1=float(a1), a3=float(a3),
        rg1=float(np.sqrt(g1)), ra=float(np.sqrt(alpha)),
    )


_W = _weights()
_NC = None


def _build_nc():
    import os
    import concourse.bacc as bacc
    import concourse.mybir as mybir
    import concourse.tile as tile

    no_accum = os.environ.get("BIOM_NO_ACCUM", "0") == "1"
    no_warm = os.environ.get("BIOM_NO_WARM", "0") == "1"

    f32 = mybir.dt.float32
    bf16 = mybir.dt.bfloat16
    Sq = mybir.ActivationFunctionType.Square
    ALU = mybir.AluOpType
    rg1, ra = _W["rg1"], _W["ra"]

    nc = bacc.Bacc()
    packed = nc.dram_tensor("packed", [P, 10 * J], bf16, kind="ExternalInput")
    out = nc.dram_tensor("out", [P, 2 * NT], f32, kind="ExternalOutput")

    with tile.TileContext(nc) as tc:
        with (
            tc.tile_pool(name="io", bufs=2) as io,
            tc.tile_pool(name="mid", bufs=2) as mid,
            tc.tile_pool(name="stats", bufs=1) as stats_pool,
        ):
            stats = stats_pool.tile([P, 2 * NT], f32)

            if not no_warm:
                # trigger the Square act-table load during the first DMA
                warm = stats_pool.tile([P, 2], bf16)
                nc.vector.memset(warm, 0.0)
                nc.scalar.activation(warm, warm, Sq)

            c0 = 0
            for t, F in enumerate(CHUNKS):
                S = io.tile([P, 4 * F], bf16, tag="S")
                C = io.tile([P, 3 * F], bf16, tag="C")
                # D1: [A4 A5 A6 | b3]
                nc.sync.dma_start(out=S[:], in_=packed[:, c0:c0 + 4 * F])
                if no_accum:
                    S2 = io.tile([P, 3 * F], bf16, tag="S2")
                    nc.sync.dma_start(
                        out=S2[:], in_=packed[:, c0 + 4 * F:c0 + 7 * F])
                    nc.vector.tensor_add(S[:, 0:3 * F], S[:, 0:3 * F], S2[:])
                else:
                    # D2: shear partners accumulated in the DMA engines
                    nc.gpsimd.dma_start(
                        out=S[:, 0:3 * F],
                        in_=packed[:, c0 + 4 * F:c0 + 7 * F],
                        accum_op=ALU.add,
                    )
                # D3: [b1 b2 | sd]
                nc.sync.dma_start(out=C[:], in_=packed[:, c0 + 7 * F:c0 + 10 * F])
                c0 += 10 * F

                X = mid.tile([P, 4 * F], bf16, tag="X")
                Z = mid.tile([P, 3 * F], bf16, tag="Z")

                # x4 x5 x6 (scale-1: sqrt(d) folded at host)
                nc.scalar.activation(X[:, 0:3 * F], S[:, 0:3 * F], Sq)
                # z3 = b3^2 (gamma3 == 1)
                nc.vector.tensor_mul(X[:, 3 * F:4 * F], S[:, 3 * F:4 * F],
                                     S[:, 3 * F:4 * F])
                # z1 z2
                nc.scalar.activation(Z[:, 0:2 * F], C[:, 0:2 * F], Sq, scale=rg1)

                # s0 = b1 + b2 + b3
                t0 = mid.tile([P, F], bf16, tag="t0")
                nc.vector.tensor_add(t0, C[:, 0:F], C[:, F:2 * F])
                s0 = mid.tile([P, F], bf16, tag="s0")
                nc.vector.tensor_add(s0, t0, S[:, 3 * F:4 * F])
                # x0 = (ra*s0)^2
                nc.scalar.activation(Z[:, 2 * F:3 * F], s0, Sq, scale=ra)

                # fold 7 terms -> q
                nc.vector.tensor_add(X[:, 0:3 * F], X[:, 0:3 * F], Z[:, 0:3 * F])
                nc.vector.tensor_add(X[:, 0:2 * F], X[:, 0:2 * F],
                                     X[:, 2 * F:4 * F])
                q = mid.tile([P, F], bf16, tag="q")
                nc.vector.tensor_add(q, X[:, 0:F], X[:, F:2 * F])

                # mask (+ fused count accum), qm = q*m
                m = mid.tile([P, F], bf16, tag="m")
                nc.vector.tensor_scalar(
                    out=m, in0=C[:, 2 * F:3 * F], scalar1=THRESH, scalar2=None,
                    op0=ALU.is_lt, op1=ALU.add,
                    accum_out=stats[:, NT + t:NT + t + 1])
                nc.vector.tensor_mul(m, q, m)
                junk = mid.tile([P, F], bf16, tag="junk")
                nc.scalar.activation(junk, m, Sq, accum_out=stats[:, t:t + 1])

            nc.sync.dma_start(out=out[:, :], in_=stats[:])

    nc.compile()
    return nc


def _get_nc():
    global _NC
    if _NC is None:
        _NC = _build_nc()
    return _NC


def _run(in_maps, trace=False, **kwargs):
    from concourse.bass_utils import run_bass_kernel_spmd

    nc = _get_nc()
    return run_bass_kernel_spmd(
        nc, in_maps, core_ids=list(range(NCORES)), trace=trace, **kwargs)


def _make_in_maps(grad_u, grad_v, grad_w, gt_sdf):
    import ml_dtypes
    bf = ml_dtypes.bfloat16

    grad_u = np.asarray(grad_u, dtype=np.float32)
    grad_v = np.asarray(grad_v, dtype=np.float32)
    grad_w = np.asarray(grad_w, dtype=np.float32)
    gt_sdf = np.asarray(gt_sdf, dtype=np.float32)
    rd = np.float32(_W["rd"]); a1 = np.float32(_W["a1"])
    a3 = np.float32(_W["a3"])

    in_maps = []
    for c in range(NCORES):
        sl = slice(c * N_LOCAL, (c + 1) * N_LOCAL)
        gu = grad_u[sl].reshape(P, J, 3)
        gv = grad_v[sl].reshape(P, J, 3)
        gw = grad_w[sl].reshape(P, J, 3)
        sd = gt_sdf[sl].reshape(P, J)
        parts = []
        off = 0
        for F in CHUNKS:
            s = slice(off, off + F)
            parts += [
                rd * gu[:, s, 1], rd * gu[:, s, 2], rd * gw[:, s, 1],  # A
                a3 * gw[:, s, 2],                                      # b3
                rd * gv[:, s, 0], rd * gw[:, s, 0], rd * gv[:, s, 2],  # B
                a1 * gu[:, s, 0], a1 * gv[:, s, 1],                    # b1 b2
                sd[:, s],
            ]
            off += F
        packed = np.ascontiguousarray(
            np.concatenate(parts, axis=1)).astype(bf)
        in_maps.append({"packed": packed})
    return in_maps


def _finalize(results):
    ssq = 0.0
    cnt = 0.0
    for res in results:
        st = np.asarray(res["out"], dtype=np.float64)
        ssq += st[:, :NT].sum()
        cnt += st[:, NT:].sum()
    return np.float32(np.sqrt(ssq) / cnt)


def kernel(grad_u, grad_v, grad_w, gt_sdf):
    in_maps = _make_in_maps(grad_u, grad_v, grad_w, gt_sdf)
    res = _run(in_maps, trace=False)
    return _finalize(res.results)


# revision 9
# speedup vs baseline: 1.3235x; 1.0920x over previous
"""Trainium2 Bass kernel for nn_BiomechanicsLoss (masked quadratic-form loss).

Math per point: et = [u0, v1, w2, .5(u1+v0), .5(u2+w0), .5(w1+v2)],
q = et^T C et (C = f32 stiffness), loss = sqrt(sum_masked(q^2)) / count,
mask = gt_sdf < 1e-8.

q decomposes (A = 3x3 normal block of sym(C), d = Cs[3,3]/4) as
  q = alpha*(b1+b2+b3)^2 + g1*(b1^2+b2^2) + 1.0*b3^2 + x4+x5+x6
with b_i = a_i*s_i (host-applied scale), x_k = d*s_k'^2 (sqrt(d) host-
applied), and alpha solved so gamma3 == 1 exactly: b3^2 then joins the
shear squares in ONE scale-1 wide Square on ScalarE.

Distribution: pure data-parallel over points, 8 cores; host packs each
core's shard to bf16 [128, 10*J] with all constant scales folded into the
quantization; each core reduces to per-chunk (ssq, count) f32 columns which
the host combines (sqrt, divide).

Per chunk of width F (points per partition), engines:
  DMA   D1 [A4 A5 A6 B4 B5 B6](6F) -> AB      D2 [b1 b2 sd b3](4F) -> C[0:4F]
  DVE   s456 = AB[0:3F]+AB[3F:6F] -> C[4F:7F]   (=> [b3|s456] contiguous)
        s0 = b1+b2+b3 (2 adds); fold X+=Z, X[0:2F]+=X[2F:4F], q = X0+X1
  ScalE X[0:4F] = Sq(C[3F:7F]) = [z3 x4 x5 x6]; Z[0:2F] = Sq(rg1*b12);
        Z[2F:3F] = Sq(ra*s0)
  Pool  qm = (sd<th)*q (scalar_tensor_tensor); count via tensor_scalar
        accum; ssq via scalar_tensor_tensor(qm*qm) accum  (all off the
        critical path)
"""

import numpy as np

N = 4_194_304
NCORES = 8
N_LOCAL = N // NCORES  # 524288
P = 128
J = N_LOCAL // P  # 4096 points per partition
CHUNKS = [640, 1216, 1216, 1024]
NT = len(CHUNKS)
assert sum(CHUNKS) == J

THRESH = 1e-8


def _weights():
    vp, Ep = 0.4, 0.21
    Ci = np.zeros((6, 6), dtype=np.float64)
    Ci[0, 0] = 1 / Ep;  Ci[0, 1] = -vp / Ep; Ci[0, 2] = -vp / Ep
    Ci[1, 0] = -vp / Ep; Ci[1, 1] = 1 / Ep;  Ci[1, 2] = -vp / Ep
    Ci[2, 0] = -vp;      Ci[2, 1] = -vp;     Ci[2, 2] = 1 / Ep
    Ci[3, 3] = 2 * (1 + vp) / Ep
    Ci[4, 4] = Ci[3, 3]
    Ci[5, 5] = Ci[3, 3]
    # match reference: inverse computed in f64, cast to f32
    C = np.linalg.inv(Ci).astype(np.float32).astype(np.float64)
    Cs = 0.5 * (C + C.T)
    A = Cs[:3, :3]
    d = 0.25 * Cs[3, 3]
    A12, A13 = A[0, 1], A[0, 2]
    # alpha s.t. gamma3 == 1 (A13 == A23, A11 == A22 for these constants)
    alpha = A13 ** 2 / (A[2, 2] * A12 - A13 ** 2)
    a1 = np.sqrt(A12 / alpha)
    a3 = a1 * A13 / A12
    g1 = A[0, 0] / a1 ** 2 - alpha
    return dict(
        rd=float(np.sqrt(d)), a1=float(a1), a3=float(a3),
        rg1=float(np.sqrt(g1)), ra=float(np.sqrt(alpha)),
    )


_W = _weights()
_NC = None


def _build_nc():
    import concourse.bacc as bacc
    import concourse.mybir as mybir
    import concourse.tile as tile

    f32 = mybir.dt.float32
    bf16 = mybir.dt.bfloat16
    Sq = mybir.ActivationFunctionType.Square
    ALU = mybir.AluOpType
    rg1, ra = _W["rg1"], _W["ra"]

    nc = bacc.Bacc()
    packed = nc.dram_tensor("packed", [P, 10 * J], bf16, kind="ExternalInput")
    out = nc.dram_tensor("out", [P, 2 * NT], f32, kind="ExternalOutput")

    with tile.TileContext(nc) as tc:
        with (
            tc.tile_pool(name="io", bufs=3) as io,
            tc.tile_pool(name="mid", bufs=2) as mid,
            tc.tile_pool(name="stats", bufs=1) as stats_pool,
        ):
            stats = stats_pool.tile([P, 2 * NT], f32)

            # trigger the Square act-table load during the first DMA
            warm = stats_pool.tile([P, 2], bf16)
            nc.gpsimd.memset(warm, 0.0)
            nc.scalar.activation(warm, warm, Sq)

            c0 = 0
            for t, F in enumerate(CHUNKS):
                AB = io.tile([P, 6 * F], bf16, tag="AB")
                C = io.tile([P, 8 * F], bf16, tag="C")
                # D1: [A4 A5 A6 | B4 B5 B6]
                nc.sync.dma_start(out=AB[:], in_=packed[:, c0:c0 + 6 * F])
                # D2: [b1 b2 | sd | b3]
                nc.sync.dma_start(out=C[:, 0:4 * F],
                                  in_=packed[:, c0 + 6 * F:c0 + 10 * F])
                c0 += 10 * F

                b12 = C[:, 0:2 * F]
                sd = C[:, 2 * F:3 * F]
                b3 = C[:, 3 * F:4 * F]

                # s456 -> C[4F:7F]: makes [b3|s4|s5|s6] one contiguous 4F run
                nc.vector.tensor_add(C[:, 4 * F:7 * F], AB[:, 0:3 * F],
                                     AB[:, 3 * F:6 * F])

                X = mid.tile([P, 4 * F], bf16, tag="X")
                Z = mid.tile([P, 3 * F], bf16, tag="Z")

                # X = [z3 x4 x5 x6]: z3/x4 on ScalarE, x5/x6 on GpSimd
                nc.scalar.activation(X[:, 0:2 * F], C[:, 3 * F:5 * F], Sq)
                nc.gpsimd.tensor_mul(X[:, 2 * F:4 * F], C[:, 5 * F:7 * F],
                                     C[:, 5 * F:7 * F])
                # z1 z2
                nc.scalar.activation(Z[:, 0:2 * F], b12, Sq, scale=rg1)

                # s0 = b1 + b2 + b3
                t0 = mid.tile([P, F], bf16, tag="t0")
                nc.vector.tensor_add(t0, C[:, 0:F], C[:, F:2 * F])
                s0 = mid.tile([P, F], bf16, tag="s0")
                nc.vector.tensor_add(s0, t0, b3)
                # x0 = (ra*s0)^2
                nc.scalar.activation(Z[:, 2 * F:3 * F], s0, Sq, scale=ra)

                # fold 7 terms -> q
                nc.vector.tensor_add(X[:, F:4 * F], X[:, F:4 * F], Z[:])
                nc.vector.tensor_add(X[:, 0:2 * F], X[:, 0:2 * F],
                                     X[:, 2 * F:4 * F])
                q = mid.tile([P, F], bf16, tag="q")
                nc.vector.tensor_add(q, X[:, 0:F], X[:, F:2 * F])

                # tail: mask (+count accum), qm = q*m, ssq accum
                m = mid.tile([P, F], bf16, tag="m")
                nc.vector.tensor_scalar(
                    out=m, in0=sd, scalar1=THRESH, scalar2=None,
                    op0=ALU.is_lt, op1=ALU.add,
                    accum_out=stats[:, NT + t:NT + t + 1])
                nc.gpsimd.tensor_mul(m, q, m)
                sj = mid.tile([P, F], bf16, tag="sj")
                nc.scalar.activation(sj, m, Sq, accum_out=stats[:, t:t + 1])

            nc.sync.dma_start(out=out[:, :], in_=stats[:])

    nc.compile()
    return nc


def _get_nc():
    global _NC
    if _NC is None:
        _NC = _build_nc()
    return _NC


def _run(in_maps, trace=False, **kwargs):
    from concourse.bass_utils import run_bass_kernel_spmd

    nc = _get_nc()
    return run_bass_kernel_spmd(
        nc, in_maps, core_ids=list(range(NCORES)), trace=trace, **kwargs)


def _make_in_maps(grad_u, grad_v, grad_w, gt_sdf):
    import ml_dtypes
    bf = ml_dtypes.bfloat16

    grad_u = np.asarray(grad_u, dtype=np.float32)
    grad_v = np.asarray(grad_v, dtype=np.float32)
    grad_w = np.asarray(grad_w, dtype=np.float32)
    gt_sdf = np.asarray(gt_sdf, dtype=np.float32)
    rd = np.float32(_W["rd"]); a1 = np.float32(_W["a1"])
    a3 = np.float32(_W["a3"])

    in_maps = []
    for c in range(NCORES):
        sl = slice(c * N_LOCAL, (c + 1) * N_LOCAL)
        gu = grad_u[sl].reshape(P, J, 3)
        gv = grad_v[sl].reshape(P, J, 3)
        gw = grad_w[sl].reshape(P, J, 3)
        sd = gt_sdf[sl].reshape(P, J)
        parts = []
        off = 0
        for F in CHUNKS:
            s = slice(off, off + F)
            parts += [
                rd * gu[:, s, 1], rd * gu[:, s, 2], rd * gw[:, s, 1],  # A
                rd * gv[:, s, 0], rd * gw[:, s, 0], rd * gv[:, s, 2],  # B
                a1 * gu[:, s, 0], a1 * gv[:, s, 1],                    # b1 b2
                sd[:, s],
                a3 * gw[:, s, 2],                                      # b3
            ]
            off += F
        packed = np.ascontiguousarray(
            np.concatenate(parts, axis=1)).astype(bf)
        in_maps.append({"packed": packed})
    return in_maps


def _finalize(results):
    ssq = 0.0
    cnt = 0.0
    for res in results:
        st = np.asarray(res["out"], dtype=np.float64)
        ssq += st[:, :NT].sum()
        cnt += st[:, NT:].sum()
    return np.float32(np.sqrt(ssq) / cnt)


def kernel(grad_u, grad_v, grad_w, gt_sdf):
    in_maps = _make_in_maps(grad_u, grad_v, grad_w, gt_sdf)
    res = _run(in_maps, trace=False)
    return _finalize(res.results)
